# revision 2
# baseline (speedup 1.0000x reference)
"""VQ codebook kernel for Trainium2 (8 NeuronCores, data-parallel over batch).

Problem: image_tensor [8192, 1024] f32, codes [8192, 1024] f32.
  indexes = argmin_k ||x_b - c_k||^2           (int32 [8192])
  out     = x + stop_grad(c[indexes] - x)      (f32 [8192, 1024], == gather + straight-through)

Strategy per core (1024 rows):
  1. Selection pass on PE in fp16: s[b,k] = 2<x,c_k> - ||c_k||^2 accumulated in
     f32 PSUM; fp16 operand rounding perturbs s by <= ~0.13 while the top-2 gap
     of the distance distribution is >= 0.008 but the *rank* of the true argmin
     under fp16 noise stays 0 (measured on this data; top-4 rerank gives margin).
     The ||c||^2 term is folded into the matmul as a K=2 extra contraction with
     an fp16 hi/lo split of -csq (error ~1e-4).
  2. Per 512-wide k-block: DVE max8/max_index -> top-8 (value, global index)
     candidates; 16 blocks -> 128 candidates per row.
  3. Finale per row: top-4 candidate values (with exact-tie dedup by excluding
     already-chosen indices), gather the 4 code rows by indirect DMA, compute
     exact f32 ||x-c||^2 on DVE+ACT, pick min with ties broken by smallest
     global index (matches jnp.argmin first-occurrence).
  4. Gather the winning row, out = x + (q - x) elementwise in f32.

Codes matrix is transposed on-device via PE transpose (fp16), streamed in two
k-halves so SBUF holds only half of C^T at a time.
"""
import sys

sys.path.insert(0, "/opt/trn_rl_repo")

from contextlib import ExitStack

import numpy as np

import concourse.bass as bass
import concourse.tile as tile
from concourse import bacc, mybir
from concourse.masks import make_identity

P = 128
B = 8192          # total rows
K = 8192          # codes
D = 1024          # embed dim
NCORES = 8
BC = B // NCORES  # rows per core = 1024
NBT = BC // P     # b-tiles per core = 8
NKB = K // 512    # k-blocks of 512 = 16
NDC = D // P      # d-chunks = 8
NKT_HALF = 32     # k-tiles of 128 per half
KHALF = K // 2    # 4096
T = 4             # rerank width
BIG = 1.0e9

F32 = mybir.dt.float32
F16 = mybir.dt.float16
I32 = mybir.dt.int32
U32 = mybir.dt.uint32
U8 = mybir.dt.uint8
AF = mybir.ActivationFunctionType
ALU = mybir.AluOpType


def build_program(nc: bass.Bass):
    x_d = nc.dram_tensor("x", [BC, D], F32, kind="ExternalInput").ap()
    codes_d = nc.dram_tensor("codes", [K, D], F32, kind="ExternalInput").ap()
    idx_d = nc.dram_tensor("idx_out", [BC, 1], I32, kind="ExternalOutput").ap()
    out_d = nc.dram_tensor("out", [BC, D], F32, kind="ExternalOutput").ap()

    with tile.TileContext(nc) as tc, ExitStack() as ctx:
        res = ctx.enter_context(tc.tile_pool(name="res", bufs=1))
        xld = ctx.enter_context(tc.tile_pool(name="xld", bufs=2))
        cld = ctx.enter_context(tc.tile_pool(name="cld", bufs=3))
        c16p = ctx.enter_context(tc.tile_pool(name="c16p", bufs=2))
        srg = ctx.enter_context(tc.tile_pool(name="srg", bufs=3))
        gth = ctx.enter_context(tc.tile_pool(name="gth", bufs=2))
        dif = ctx.enter_context(tc.tile_pool(name="dif", bufs=2))
        outr = ctx.enter_context(tc.tile_pool(name="outr", bufs=2))
        fin = ctx.enter_context(tc.tile_pool(name="fin", bufs=2))
        mmp = ctx.enter_context(tc.tile_pool(name="mmp", bufs=5, space="PSUM"))
        tpp = ctx.enter_context(tc.tile_pool(name="tpp", bufs=2, space="PSUM"))

        ident = res.tile([P, P], F16)
        make_identity(nc, ident[:])
        ones2 = res.tile([2, P], F16)
        nc.vector.memset(ones2[:], 1.0)
        bigt = res.tile([P, P], F32)
        nc.vector.memset(bigt[:], BIG)

        # ---------------- phase 0: x -> fp16(2x) transposed ----------------
        x2T = [res.tile([P, BC], F16, name=f"x2T{dc}") for dc in range(NDC)]
        for bt in range(NBT):
            xt = xld.tile([P, D], F32, tag="xt")
            nc.sync.dma_start(xt[:], x_d[bass.ts(bt, P), :])
            x2 = c16p.tile([P, D], F16, tag="x2")
            nc.scalar.activation(out=x2[:], in_=xt[:], func=AF.Copy, scale=2.0)
            for dc in range(NDC):
                pt = tpp.tile([P, P], F16, space="PSUM", tag="tp")
                nc.tensor.transpose(
                    out=pt[:], in_=x2[:, bass.ts(dc, P)], identity=ident[:]
                )
                nc.scalar.activation(
                    out=x2T[dc][:, bass.ts(bt, P)], in_=pt[:], func=AF.Copy
                )

        # residents for selection
        cT = [res.tile([P, KHALF], F16, name=f"cT{dc}") for dc in range(NDC)]
        csq_nat = res.tile([P, K // P], F32)
        rhs_extra = res.tile([2, KHALF], F16)
        vals = [res.tile([P, NKB * 8], F32, name=f"vals{bt}") for bt in range(NBT)]
        idxf = [res.tile([P, NKB * 8], F32, name=f"idxf{bt}") for bt in range(NBT)]

        def build_half(h: int):
            # stream codes rows [h*4096, (h+1)*4096) -> cT (fp16, transposed),
            # csq_nat cols [h*32, h*32+32)
            for kt in range(NKT_HALF):
                kt_g = h * NKT_HALF + kt
                ct_ = cld.tile([P, D], F32, tag="ct")
                nc.sync.dma_start(ct_[:], codes_d[bass.ts(kt_g, P), :])
                c16 = c16p.tile([P, D], F16, tag="c16")
                nc.gpsimd.tensor_copy(out=c16[:], in_=ct_[:])
                # in-place square (destroys ct_) + accumulate csq
                nc.scalar.activation(
                    out=ct_[:], in_=ct_[:], func=AF.Square,
                    accum_out=csq_nat[:, kt_g : kt_g + 1],
                )
                for dc in range(NDC):
                    pt = tpp.tile([P, P], F16, space="PSUM", tag="tp")
                    nc.tensor.transpose(
                        out=pt[:], in_=c16[:, bass.ts(dc, P)], identity=ident[:]
                    )
                    nc.scalar.activation(
                        out=cT[dc][:, bass.ts(kt, P)], in_=pt[:], func=AF.Copy
                    )
            # -csq hi/lo fp16 rows for this half
            csq_h = csq_nat[:, h * NKT_HALF : (h + 1) * NKT_HALF]
            hi16 = fin.tile([P, NKT_HALF], F16, tag="hi16")
            nc.scalar.activation(out=hi16[:], in_=csq_h, func=AF.Copy, scale=-1.0)
            hi32 = fin.tile([P, NKT_HALF], F32, tag="hi32")
            nc.vector.tensor_copy(out=hi32[:], in_=hi16[:])
            lo32 = fin.tile([P, NKT_HALF], F32, tag="lo32")
            nc.vector.tensor_scalar_mul(lo32[:], csq_h, -1.0)
            nc.vector.tensor_sub(lo32[:], lo32[:], hi32[:])
            lo16 = fin.tile([P, NKT_HALF], F16, tag="lo16")
            nc.vector.tensor_copy(out=lo16[:], in_=lo32[:])
            for row, nat in ((0, hi16), (1, lo16)):
                ptc = tpp.tile([NKT_HALF, P], F16, space="PSUM", tag="tp")
                nc.tensor.transpose(out=ptc[:], in_=nat[:], identity=ident[:])
                st = fin.tile([NKT_HALF, P], F16, tag="csqT")
                nc.vector.tensor_copy(out=st[:], in_=ptc[:])
                nc.sync.dma_start(
                    out=rhs_extra[row : row + 1, :].rearrange(
                        "p (t q) -> p t q", t=NKT_HALF
                    ),
                    in_=st[:],
                )

        def select_block(bt: int, h: int, kb: int):
            kb_g = h * (NKB // 2) + kb
            psum = mmp.tile([P, 512], F32, space="PSUM", tag="mm")
            for dc in range(NDC):
                nc.tensor.matmul(
                    out=psum[:],
                    lhsT=x2T[dc][:, bass.ts(bt, P)],
                    rhs=cT[dc][:, bass.ts(kb, 512)],
                    start=(dc == 0),
                    stop=False,
                )
            nc.tensor.matmul(
                out=psum[:], lhsT=ones2[:], rhs=rhs_extra[:, bass.ts(kb, 512)],
                start=False, stop=True,
            )
            s_sb = srg.tile([P, 512], F32, tag="s")
            nc.scalar.activation(out=s_sb[:], in_=psum[:], func=AF.Copy)
            c8 = slice(kb_g * 8, kb_g * 8 + 8)
            nc.vector.max(out=vals[bt][:, c8], in_=s_sb[:])
            i8u = fin.tile([P, 8], U32, tag="i8u")
            nc.vector.max_index(out=i8u[:], in_max=vals[bt][:, c8], in_values=s_sb[:])
            i8f = fin.tile([P, 8], F32, tag="i8f")
            nc.vector.tensor_copy(out=i8f[:], in_=i8u[:])
            nc.vector.tensor_scalar_add(idxf[bt][:, c8], i8f[:], float(kb_g * 512))

        def finale(bt: int):
            vb, ib = vals[bt][:], idxf[bt][:]
            f8 = fin.tile([P, 8], F32, tag="f8")
            nc.vector.max(out=f8[:], in_=vb)
            xt = xld.tile([P, D], F32, tag="xt")
            nc.sync.dma_start(xt[:], x_d[bass.ts(bt, P), :])
            d4 = fin.tile([P, T], F32, tag="d4")
            i4 = fin.tile([P, T], F32, tag="i4")
            sel = fin.tile([P, NKB * 8], F32, tag="sel")
            eq = fin.tile([P, NKB * 8], U8, tag="eq")
            ne = fin.tile([P, NKB * 8], U8, tag="ne")
            for j in range(T):
                nc.vector.tensor_scalar(
                    out=eq[:], in0=vb, scalar1=f8[:, j : j + 1], scalar2=None,
                    op0=ALU.is_equal,
                )
                for m in range(j):
                    nc.vector.tensor_scalar(
                        out=ne[:], in0=ib, scalar1=i4[:, m : m + 1], scalar2=None,
                        op0=ALU.not_equal,
                    )
                    nc.vector.tensor_tensor(
                        out=eq[:], in0=eq[:], in1=ne[:], op=ALU.mult
                    )
                nc.vector.select(
                    out=sel[:], mask=eq[:], on_true=ib, on_false=bigt[:]
                )
                nc.vector.tensor_reduce(
                    out=i4[:, j : j + 1], in_=sel[:], axis=mybir.AxisListType.X,
                    op=ALU.min,
                )
                ji = fin.tile([P, 1], I32, tag="ji")
                nc.vector.tensor_copy(out=ji[:], in_=i4[:, j : j + 1])
                g = gth.tile([P, D], F32, tag="g")
                nc.gpsimd.indirect_dma_start(
                    out=g[:], out_offset=None, in_=codes_d[:],
                    in_offset=bass.IndirectOffsetOnAxis(ap=ji[:, :1], axis=0),
                )
                df = dif.tile([P, D], F32, tag="df")
                nc.vector.tensor_sub(df[:], xt[:], g[:])
                nc.scalar.activation(
                    out=df[:], in_=df[:], func=AF.Square,
                    accum_out=d4[:, j : j + 1],
                )
            # pick min exact distance; break ties by smallest global index
            dmin = fin.tile([P, 1], F32, tag="dmin")
            nc.vector.tensor_reduce(
                out=dmin[:], in_=d4[:], axis=mybir.AxisListType.X, op=ALU.min
            )
            eq4 = fin.tile([P, T], U8, tag="eq4")
            nc.vector.tensor_scalar(
                out=eq4[:], in0=d4[:], scalar1=dmin[:, :1], scalar2=None,
                op0=ALU.is_equal,
            )
            sel4 = fin.tile([P, T], F32, tag="sel4")
            nc.vector.select(
                out=sel4[:], mask=eq4[:], on_true=i4[:], on_false=bigt[:, :T]
            )
            iwin = fin.tile([P, 1], F32, tag="iwin")
            nc.vector.tensor_reduce(
                out=iwin[:], in_=sel4[:], axis=mybir.AxisListType.X, op=ALU.min
            )
            iwin_i = fin.tile([P, 1], I32, tag="iwin_i")
            nc.vector.tensor_copy(out=iwin_i[:], in_=iwin[:])
            nc.sync.dma_start(idx_d[bass.ts(bt, P), :], iwin_i[:])
            # gather winner, straight-through out = x + (q - x)
            q = gth.tile([P, D], F32, tag="g")
            nc.gpsimd.indirect_dma_start(
                out=q[:], out_offset=None, in_=codes_d[:],
                in_offset=bass.IndirectOffsetOnAxis(ap=iwin_i[:, :1], axis=0),
            )
            ob = outr.tile([P, D], F32, tag="ob")
            nc.gpsimd.tensor_sub(ob[:], q[:], xt[:])
            nc.gpsimd.tensor_add(ob[:], ob[:], xt[:])
            nc.sync.dma_start(out_d[bass.ts(bt, P), :], ob[:])

        # ---------------- main ----------------
        for h in range(2):
            build_half(h)
            for bt in range(NBT):
                for kb in range(NKB // 2):
                    select_block(bt, h, kb)
                if h == 1:
                    finale(bt)

    return nc


_CACHED = None


def _get_nc():
    global _CACHED
    if _CACHED is None:
        nc = bacc.Bacc(
            "TRN2", target_bir_lowering=False, debug=False, num_devices=NCORES
        )
        build_program(nc)
        nc.compile()
        _CACHED = nc
    return _CACHED


def kernel(image_tensor, codes):
    from concourse.bass_utils import run_bass_kernel_spmd

    x = np.ascontiguousarray(np.asarray(image_tensor, dtype=np.float32))
    c = np.ascontiguousarray(np.asarray(codes, dtype=np.float32))
    assert x.shape == (B, D) and c.shape == (K, D)

    nc = _get_nc()
    in_maps = [
        {"x": x[i * BC : (i + 1) * BC], "codes": c} for i in range(NCORES)
    ]
    res = run_bass_kernel_spmd(nc, in_maps, core_ids=list(range(NCORES)))
    idx = np.concatenate(
        [r["idx_out"].reshape(BC) for r in res.results]
    ).astype(np.int32)
    out = np.concatenate([r["out"] for r in res.results], axis=0)
    return idx, out


if __name__ == "__main__":
    # smoke: build + compile only
    _get_nc()
    print("build+compile OK")
